# revision 12
# baseline (speedup 1.0000x reference)
"""FlowNetC-style SpatialCorrelationSampler (max_disp=20, dilation_patch=2)
as a Bass/Tile kernel for 8 Trainium2 NeuronCores.

Strategy
-------
Data-parallel over batch: core i handles sample i (B == 8 == n_cores).

Per core (C=256, H=64, W=96, 441 displacement channels, all even):
displacements are even in both axes, so the problem splits into 4
independent phase sub-problems over the (y%2, x%2) sub-grids (32 x 48),
each with sub-displacements (dy', dx') in [-10, 10]^2.

Both phase images are stored with a zero-padded row pitch of 58
(x in [-10, 58), adjacent rows share the 10-wide zero gap), flat
position n = 58*y + x + 10, so out-of-range dx' reads hit exact zeros.
For each phase the TensorEngine computes the full Gram matrix
G[m, n] = sum_c x2p[c, m] * x1p[c, n] (bf16 in, fp32 PSUM), evacuated
to a bf16 slab with chunk-major layout
slab[p, px*PXS + k*NP + n] = G_px[128k + p, n].

The outputs are the diagonals out[di, dj, y, x] = G[n + s - 590, n],
s_idx = 58*di + dj in [0, 1181).  Diagonal extraction is a
per-partition shear that no SBUF engine can address (engine access
patterns must start at partition 0/32/64/96), so it runs on the
TensorEngine: columns {n0 + 128k} share one shear offset, and a matmul
whose stationary operand is a shifted identity E[:, a:a+128] (ones on
diagonal offset a-128) shifts slab partitions by an arbitrary amount.
A 3D moving-operand access pattern walks the k-staircase for BOTH px
phases at once (the px sub-slabs sit at a constant stride), so one
matmul extracts a [128, 2*kc] brick.  Two shifts t per n0 tile each
128-row sigma window exactly; rows outside a shift's E-window are
written as zeros, which zero-fills out-of-range dy' automatically.

PSUM (4 quarter tiles of 2 banks per sigma window, column-grouped 32
per residue n0) is evacuated straight into the x-interleaved output
layout (uniform stride-2 map 2n+px), never-written "stale" column
ranges are zeroed with batched full-partition memsets, and
d-contiguous row runs are DMA'd out per (sigma, di) (DMA has no
partition-alignment restriction).
"""

import sys

for _p in ("/opt/trn_rl_repo",):
    if _p not in sys.path:
        sys.path.insert(0, _p)

import numpy as np

import concourse.bass as bass
import concourse.tile as tile
from concourse import bacc, mybir
from concourse.bass_utils import run_bass_kernel_spmd

F32 = mybir.dt.float32
BF16 = mybir.dt.bfloat16

B = 8
C = 256
H = 64
W = 96
HS, WS = H // 2, W // 2      # 32 x 48 sub-grid
R = 10                       # sub-displacement radius
P = 2 * R + 1                # 21
D = P * P                    # 441 output channels

WP = WS + R                  # 58: padded row pitch (shared 10-wide zero gap)
XOFF = R                     # x offset inside a padded row
NP = WP * (HS - 1) + WS + 2 * R  # 1866 flat positions (incl. pads)
NK = 15                      # 128-row m-chunks covering [0, 1866)
NPAD = NK * 128              # 1920: zero-padded input width (m side)
S0 = WP * R + XOFF           # 590:  s_idx = (m - n) + S0 = 58*di + dj
SIDX = WP * (P - 1) + P      # 1181 used diagonal offsets
NSIG = (SIDX + 127) // 128   # 10 sigma windows
KSTEP = NP + 128             # 1994: slab free step of the k-batch staircase
PXS = NK * NP                # 27990: px sub-slab stride


def _t_list(n0):
    """Shifts t with Delta = 128t + n0 - S0 in (-128, 128)."""
    return [t for t in range(0, 12) if -128 < 128 * t + n0 - S0 < 128]


def _krange(sig, t, n0):
    """k-interval whose staircase copy is in-bounds for this shift."""
    d = sig - t
    kmax = NK if n0 < NP - 128 * (NK - 1) else NK - 1  # n0+128k < NP
    return max(0, -d), min(kmax, NK - d)


def _isect(a, b):
    lo, hi = max(a[0], b[0]), min(a[1], b[1])
    return (lo, hi) if lo < hi else None


def _diff(a, b):
    out = []
    if a[0] < min(a[1], b[0]):
        out.append((a[0], min(a[1], b[0])))
    if max(a[0], b[1]) < a[1]:
        out.append((max(a[0], b[1]), a[1]))
    return out


def _n0_regimes():
    """n0 ranges sharing the same t-list and k-cap."""
    bounds, prev = set(), None
    for n0 in range(128):
        key = (tuple(_t_list(n0)), n0 < NP - 128 * (NK - 1))
        if key != prev:
            bounds.add(n0)
            prev = key
    bs = sorted(bounds) + [128]
    return [(bs[i], bs[i + 1]) for i in range(len(bs) - 1)]


def _build_program(loop_n=None):
    nc = bacc.Bacc("TRN2", target_bir_lowering=False, debug=False)

    x1d = nc.dram_tensor("x1", [C, H, W], F32, kind="ExternalInput").ap()
    x2d = nc.dram_tensor("x2", [C, H, W], F32, kind="ExternalInput").ap()
    outd = nc.dram_tensor("out", [D, H, W], F32, kind="ExternalOutput").ap()

    with tile.TileContext(nc) as tc:
        if loop_n is None:
            _corr_kernel(tc, outd, x1d, x2d)
        else:
            with tc.For_i(0, loop_n, 1):
                _corr_kernel(tc, outd, x1d, x2d)
    nc.compile()
    return nc


def _corr_kernel(tc, outd, x1d, x2d):
    from contextlib import ExitStack

    nc = tc.nc
    with ExitStack() as ctx:
        const_pool = ctx.enter_context(tc.tile_pool(name="const", bufs=1))
        stage_pool = ctx.enter_context(tc.tile_pool(name="stage", bufs=1))
        pack_pool = ctx.enter_context(tc.tile_pool(name="pack", bufs=1))
        slab_pool = ctx.enter_context(tc.tile_pool(name="slab", bufs=1))
        outb_pool = ctx.enter_context(tc.tile_pool(name="outb", bufs=2))
        psum_pool = ctx.enter_context(
            tc.tile_pool(name="psum", bufs=1, space="PSUM"))

        # --- shifted-identity strip: E[p, f] = 1 iff f - p == 128 ---
        ones = stage_pool.tile([128, 384], BF16, tag="stage")
        nc.vector.memset(ones[:, :], 1.0)
        E = const_pool.tile([128, 384], BF16)
        nc.gpsimd.affine_select(
            E[:, :], ones[:, :], pattern=[[1, 384]], base=-128,
            channel_multiplier=-1,
            compare_op=mybir.AluOpType.is_equal, fill=0.0)

        n_bal = 0  # round-robin DVE/ACT for copies

        def copy_op(dst, src):
            nonlocal n_bal
            if n_bal % 2:
                nc.scalar.copy(dst, src)
            else:
                nc.vector.tensor_copy(dst, src)
            n_bal += 1

        for py in (0, 1):
            # ---- load + phase-pack inputs into padded bf16 layout ----
            packed = {}
            for t_idx, src in ((0, x1d), (1, x2d)):
                for cc in (0, 1):
                    stg = stage_pool.tile([128, HS * W], F32, tag="stage",
                                          name=f"stg_{py}_{t_idx}_{cc}")
                    nc.sync.dma_start(
                        stg.rearrange("p (y x) -> p y x", x=W),
                        src[cc * 128:(cc + 1) * 128, py::2, :])
                    stg_v = stg.rearrange("p (y x) -> p y x", x=W)
                    for px in (0, 1):
                        pk = pack_pool.tile(
                            [128, NPAD], BF16, tag=f"pk{t_idx}{cc}{px}",
                            name=f"pk_{py}_{t_idx}_{cc}_{px}")
                        nc.vector.memset(pk[:, :], 0.0)
                        dstv = bass.AP(pk.tensor, XOFF,
                                       [[pk.ap[0][0], 128], [WP, HS], [1, WS]])
                        nc.vector.tensor_copy(dstv, stg_v[:, :, px::2])
                        packed[(t_idx, cc, px)] = pk

            # ---- Gram for both px phases -> one bf16 slab ----
            slab = slab_pool.tile([128, 2 * PXS], BF16, tag="slab",
                                  name=f"slab_{py}")
            rp_s = slab.ap[0][0]
            n_ps = 0
            for px in (0, 1):
                for k in range(NK):
                    for nb in range(4):
                        nbw = min(512, NP - nb * 512)
                        ps = psum_pool.tile([128, 1024], F32,
                                            tag=f"ps{n_ps % 4}",
                                            name=f"psg_{py}_{px}_{k}_{nb}")
                        n_ps += 1
                        for cc in (0, 1):
                            nc.tensor.matmul(
                                ps[:, 0:nbw],
                                lhsT=packed[(1, cc, px)][:, 128 * k:128 * (k + 1)],
                                rhs=packed[(0, cc, px)][:, 512 * nb:512 * nb + nbw],
                                start=(cc == 0), stop=(cc == 1))
                        base = px * PXS + k * NP + nb * 512
                        copy_op(slab[:, base:base + nbw], ps[:, 0:nbw])

            # ---- per sigma window: extract diagonals on PE, interleave,
            #      zero stale, DMA ----
            for sig in range(NSIG):
                ob = outb_pool.tile([128, 2 * NPAD], F32, tag="outb",
                                    name=f"outb_{py}_{sig}")
                rp_o = ob.ap[0][0]
                for q in range(4):
                    ps = psum_pool.tile([128, 1024], F32, tag=f"ps{q}",
                                        name=f"pse_{py}_{sig}_{q}")
                    pp = ps.ap[0][0]
                    # pre-zero stale (never-written) column ranges
                    for (g0, g1) in _n0_regimes():
                        g0q, g1q = max(g0, 32 * q), min(g1, 32 * q + 32)
                        if g0q >= g1q:
                            continue
                        cov = None
                        for t in _t_list(g0q):
                            rng = _krange(sig, t, g0q)
                            if rng[0] < rng[1]:
                                cov = rng if cov is None else (
                                    min(cov[0], rng[0]), max(cov[1], rng[1]))
                        stale = []
                        if cov is None:
                            stale = [(0, NK)]
                        else:
                            if cov[0] > 0:
                                stale.append((0, cov[0]))
                            if cov[1] < NK:
                                stale.append((cov[1], NK))
                        for (s_lo, s_hi) in stale:
                            zap = bass.AP(
                                ps.tensor,
                                (g0q - 32 * q) * 32 + 2 * s_lo,
                                [[pp, 128], [32, g1q - g0q],
                                 [1, 2 * (s_hi - s_lo)]])
                            nc.vector.memset(zap, 0.0)

                    # collect extraction matmuls; mark bank first/last
                    descs = []  # (a, d, klo, kc, c0)
                    for n0 in range(32 * q, 32 * q + 32):
                        ts = _t_list(n0)
                        for ti, t in enumerate(ts):
                            a = 128 * t + n0 - S0 + 128
                            rng = _krange(sig, t, n0)
                            if rng[0] >= rng[1]:
                                continue
                            if ti == 0:
                                pieces = [rng]
                            else:
                                orng = _krange(sig, ts[0], n0)
                                if orng[0] >= orng[1]:
                                    pieces = [rng]
                                else:
                                    pieces = _diff(rng, orng)
                                    ov = _isect(rng, orng)
                                    if ov:
                                        pieces.append(ov)
                            d = sig - t
                            gl = n0 - 32 * q
                            for (lo, hi) in pieces:
                                descs.append(
                                    (a, d, lo, hi - lo,
                                     gl * 32 + 2 * lo, n0))
                    first, last = {}, {}
                    for i, (_, _, _, _, c0, _) in enumerate(descs):
                        bank = c0 // 512
                        first.setdefault(bank, i)
                        last[bank] = i
                    firsts = set(first.values())
                    lasts = set(last.values())
                    for i, (a, d, klo, kc, c0, n0) in enumerate(descs):
                        rhs = bass.AP(
                            slab.tensor, NP * d + n0 + KSTEP * klo,
                            [[rp_s, 128], [KSTEP, kc], [PXS, 2]])
                        nc.tensor.matmul(
                            ps[:, c0:c0 + 2 * kc],
                            lhsT=E[:, a:a + 128], rhs=rhs,
                            start=(i in firsts), stop=(i in lasts))

                    # evac psum quarter -> interleaved outb
                    src = bass.AP(ps.tensor, 0,
                                  [[pp, 128], [32, 32], [2, NK], [1, 2]])
                    dst = bass.AP(ob.tensor, 64 * q,
                                  [[rp_o, 128], [2, 32], [256, NK], [1, 2]])
                    copy_op(dst, src)

                # ---- DMA d-contiguous runs out ----
                sig_lo = 128 * sig
                sig_hi = min(sig_lo + 128, SIDX)
                for di in range(P):
                    r_lo = max(WP * di, sig_lo)
                    r_hi = min(WP * di + P, sig_hi)
                    if r_lo >= r_hi:
                        continue
                    dj0, dj1 = r_lo - WP * di, r_hi - WP * di
                    d0 = P * di + dj0
                    src = bass.AP(
                        ob.tensor,
                        (r_lo - sig_lo) * rp_o + 2 * XOFF,
                        [[rp_o, r_hi - r_lo], [2 * WP, HS], [1, W]])
                    nc.sync.dma_start(
                        outd[d0:d0 + (dj1 - dj0), py::2, :], src)


_PROGRAM = None


def _get_program():
    global _PROGRAM
    if _PROGRAM is None:
        _PROGRAM = _build_program()
    return _PROGRAM


def kernel(x1: np.ndarray, x2: np.ndarray) -> np.ndarray:
    x1 = np.ascontiguousarray(np.asarray(x1, dtype=np.float32))
    x2 = np.ascontiguousarray(np.asarray(x2, dtype=np.float32))
    assert x1.shape == (B, C, H, W) and x2.shape == (B, C, H, W)
    nc = _get_program()
    in_maps = [{"x1": x1[i], "x2": x2[i]} for i in range(B)]
    res = run_bass_kernel_spmd(nc, in_maps, core_ids=list(range(B)))
    return np.stack([res.results[i]["out"] for i in range(B)], axis=0)


if __name__ == "__main__":
    rng = np.random.default_rng(0)
    x1 = rng.standard_normal((B, C, H, W), dtype=np.float32)
    x2 = rng.standard_normal((B, C, H, W), dtype=np.float32)
    out = kernel(x1, x2)
    print(out.shape, out.dtype, float(np.abs(out).max()))
